# revision 14
# baseline (speedup 1.0000x reference)
"""GNN edge-softmax attention kernel for 8 Trainium2 NeuronCores.

Math: logit[e] = src[e]@(W_src@a) + dest[e]@(W_dest@a) + ea[e]@(W_edge@a)
      s = leaky_relu(logit, 0.2); val = exp(s)
      out[e] = val[e] / (sum_{e' in dest-segment} val[e'] + eps)

Design (single SPMD program, identical on all 8 cores):
  * Fold the three projection matrices with the attention vector on host
    -> one 288-dim dot per edge against a fixed folded vector v.
  * |v|-striped dtypes: the logit error from quantizing dim i scales
    with |v_i|, so the 256 lowest-|v| dims stream as fp8-e3m4 (TRN2 PE
    handles e3m4 subnormals exactly) and only the top 32 dims as fp16.
    Measured softmax rel err ~1.4e-2 vs the 2e-2 gate; DMA drops to 56%
    of an all-fp16 stream (~93us at 358 GB/s/core).
  * The PE runs UNCHAINED single matmuls at ~22 cycles (measured); PSUM
    accumulation chains cost ~95c per chained matmul.  So each slot
    column issues 3 independent matmuls (fp8 128 rows, fp8 128 rows,
    fp16 32 rows) into 3 separate PSUM tiles; the partials are summed on
    DVE, which also applies the leaky relu (PE ~40us, DVE ~30us, both in
    the DMA shadow).
  * Host sorts nodes by degree and packs them into chunks of 128 nodes
    (one node per SBUF/PSUM partition).  All edges of a node live in one
    partition, padded along the free dim to the chunk max degree D_j
    (2.5% padding).  Global chunk 8j+c goes to core c as its chunk j, so
    every core has the SAME D_j list -> one program for all cores.
  * Matmuls put the DATA stationary (lhsT = [dims, 128 slots]) and the
    folded-vector column (fp16, mixed-dtype operands) moving, so logits
    land directly in node-major PSUM layout [128 nodes, D_j]: segment
    softmax collapses to per-partition row ops (ACT Exp with accum_out
    row-sum -> DVE reciprocal -> ACT scaled copy).
  * DMAs are issued per ~64-column super-group so HBM streaming and
    compute pipeline cleanly.
  * Pad slots are zero in the fp8 streams and carry a special fp16
    column that forces logit = -200 (exp -> 0), so pads never
    contaminate segment sums.
"""

import math
import os
import time

import numpy as np

import sys
sys.path.insert(0, "/opt/trn_rl_repo")

P = 128
NCORES = 8
NDIM = 288            # 128 src + 128 dest + 32 ea
NA = 128              # fp8 stream A rows (lowest |v|)
NB = 128              # fp8 stream B rows
NC_ = NDIM - NA - NB  # fp16 stream C rows (highest |v|)
NEG_SLOPE = 0.2
EPS = 1e-16
PAD_LOGIT = -200.0

LAST_EXEC_NS = None
LAST_WALL_NS = None

_CACHE = {}


# --------------------------------------------------------------------------- #
# Host-side preparation
# --------------------------------------------------------------------------- #

def _host_prep(src, dest, edge_attr, col, n_nodes, v_full):
    """Degree-sorted node-major padded layout with |v|-striped dtypes."""
    import ml_dtypes
    fp8 = ml_dtypes.float8_e3m4

    E = src.shape[0]
    N = n_nodes
    n_groups = math.ceil(N / (NCORES * P))          # chunk slots per core
    NPAD = n_groups * NCORES * P

    deg = np.bincount(col, minlength=N).astype(np.int64)
    deg_ext = np.zeros(NPAD, np.int64)
    deg_ext[:N] = deg
    start_ext = np.zeros(NPAD, np.int64)
    start_ext[:N] = np.concatenate([[0], np.cumsum(deg)[:-1]])
    perm = np.argsort(col, kind="stable")           # edges sorted by dest

    order = np.argsort(deg_ext, kind="stable")      # nodes by degree (asc)

    # D per chunk-slot j (shared across cores): max degree in group of 8 chunks
    order_mat = order.reshape(n_groups, NCORES, P)  # [j, core, p]
    deg_mat = deg_ext[order_mat]                    # [j, core, p]
    D_list = deg_mat.max(axis=(1, 2)).astype(np.int64)   # [j]
    keep = D_list > 0
    C = int(D_list.sum())

    # slot -> edge map per core: M[c][p, cg] with cg = B_j + k
    B = np.concatenate([[0], np.cumsum(D_list)[:-1]])
    M_edge = np.full((NCORES, P, C), -1, np.int64)
    for j in range(n_groups):
        D = int(D_list[j])
        if D == 0:
            continue
        b = int(B[j])
        ns = order_mat[j]                           # [core, p]
        degs = deg_ext[ns][:, :, None]              # [core, p, 1]
        sts = start_ext[ns][:, :, None]
        ks = np.arange(D)[None, None, :]            # [1, 1, D]
        valid = ks < degs
        eidx = np.where(valid, sts + ks, 0)
        eids = np.where(valid, perm[eidx], -1)      # [core, p, D]
        M_edge[:, :, b:b + D] = eids

    S = C * P
    # flat slot s = cg*P + p  -> edge id
    slot_edge = M_edge.transpose(0, 2, 1).reshape(NCORES, S)  # [c, s]

    # |v|-sorted dim split: lowest NA+NB -> fp8; top NC_ -> fp16.
    o = np.argsort(np.abs(v_full), kind="stable")
    dims_a = o[:NA]
    dims_b = o[NA:NA + NB]
    dims_c = o[NA + NB:]

    v_a = v_full[dims_a].astype(np.float16)
    v_b = v_full[dims_b].astype(np.float16)
    v_c = v_full[dims_c].astype(np.float16)

    # pad column in the fp16 stream forces logit = PAD_LOGIT
    vcf = v_c.astype(np.float32)
    alpha = PAD_LOGIT / float(np.dot(vcf, vcf))
    padc = (alpha * vcf).astype(np.float16)
    lp = float(np.dot(padc.astype(np.float32), vcf))
    padc = (padc.astype(np.float32) * (PAD_LOGIT / lp)).astype(np.float16)

    xa = np.zeros((NCORES, NA, S), fp8)
    xb = np.zeros((NCORES, NB, S), fp8)
    xc = np.empty((NCORES, NC_, S), np.float16)
    for c in range(NCORES):
        se = slot_edge[c]
        m = se >= 0
        ids = se[m]
        big = np.concatenate(
            [src[ids], dest[ids], edge_attr[ids]], axis=1)  # [e_c, 288] f32
        ta = np.zeros((S, NA), np.float32)
        ta[m] = big[:, dims_a]
        xa[c] = ta.T.astype(fp8)
        tb = np.zeros((S, NB), np.float32)
        tb[m] = big[:, dims_b]
        xb[c] = tb.T.astype(fp8)
        tcc = np.empty((S, NC_), np.float32)
        tcc[:] = padc[None, :]
        tcc[m] = big[:, dims_c]
        xc[c] = tcc.T.astype(np.float16)

    cst = np.zeros((P, 3), np.float16)
    cst[:NA, 0] = v_a
    cst[:NB, 1] = v_b
    cst[:NC_, 2] = v_c

    return dict(D_list=D_list[keep].tolist(), C=C, S=S,
                slot_edge=slot_edge, xa=xa, xb=xb, xc=xc, cst=cst)


# --------------------------------------------------------------------------- #
# Device program (one program, all cores)
# --------------------------------------------------------------------------- #

GROUP_COLS = 100  # DMA super-group budget (columns)


def _make_groups(D_list):
    """Greedy-group consecutive chunks with total columns <= budget.

    The first groups are small (8/16/32 cols): each For_i iteration starts
    behind an all-engine barrier, so the first group's DMA latency is fully
    exposed — a short ramp lets the PE start ~5us earlier."""
    budget = max(GROUP_COLS, max(D_list))
    groups = []
    cur, tot = [], 0
    for j, D in enumerate(D_list):
        if cur and tot + D > budget:
            groups.append(cur)
            cur, tot = [], 0
        cur.append(j)
        tot += D
    if cur:
        groups.append(cur)
    return groups, budget


def _build_program(D_list, C, n_iter=1):
    from concourse import bacc, mybir
    from concourse import tile
    import contextlib

    f32 = mybir.dt.float32
    f16 = mybir.dt.float16
    f8 = mybir.dt.float8e3
    AF = mybir.ActivationFunctionType
    OP = mybir.AluOpType
    S = C * P
    D_max = max(D_list)
    groups, budget = _make_groups(D_list)
    B = np.concatenate([[0], np.cumsum(D_list)]).astype(int)

    nc = bacc.Bacc("TRN2", target_bir_lowering=False, debug=True)

    xa = nc.declare_dram_parameter("xa", [NA, S], f8, isOutput=False)
    xb = nc.declare_dram_parameter("xb", [NB, S], f8, isOutput=False)
    xc = nc.declare_dram_parameter("xc", [NC_, S], f16, isOutput=False)
    xcst = nc.declare_dram_parameter("xcst", [P, 3], f16, isOutput=False)
    yout = nc.declare_dram_parameter("yout", [P, C], f16, isOutput=True)

    with tile.TileContext(nc) as tc:
        with (
            tc.tile_pool(name="consts", bufs=1) as cpool,
            tc.tile_pool(name="stream", bufs=3) as spool,
            tc.tile_pool(name="tmp", bufs=4) as tpool,
            tc.tile_pool(name="outbuf", bufs=1) as opool,
            tc.tile_pool(name="ps", bufs=6, space="PSUM") as pspool,
        ):
            loop = (tc.For_i(0, n_iter) if n_iter > 1
                    else contextlib.nullcontext())
            with loop:
                cst = cpool.tile([P, 3], f16, tag="cst")
                nc.sync.dma_start(out=cst[:], in_=xcst[:])
                rva = cst[0:NA, 0:1]
                rvb = cst[0:NB, 1:2]
                rvc = cst[0:NC_, 2:3]

                out_sb = opool.tile([P, C], f16, tag="out_sb")

                for g in groups:
                    g0, g1 = B[g[0]], B[g[-1] + 1]
                    W = int(g1 - g0)
                    ba = spool.tile([NA, budget * P], f8, tag="ba")
                    bb = spool.tile([NB, budget * P], f8, tag="bb")
                    bc = spool.tile([NC_, budget * P], f16, tag="bc")
                    nc.sync.dma_start(out=ba[:, :W * P],
                                      in_=xa[:, g0 * P:g1 * P])
                    nc.sync.dma_start(out=bb[:, :W * P],
                                      in_=xb[:, g0 * P:g1 * P])
                    nc.sync.dma_start(out=bc[:, :W * P],
                                      in_=xc[:, g0 * P:g1 * P])

                    for j in g:
                        D = int(D_list[j])
                        b = int(B[j])
                        o = b - int(g0)          # column offset inside group
                        # one PSUM bank per chunk holds all 3 partials; 6-deep
                        # rotation decouples the PE from the combine tail
                        ps = pspool.tile([P, 3 * D_max], f32, tag="ps")
                        ob_, oc_ = D_max, 2 * D_max
                        # One run per stream: PE tile-config switches (128-row
                        # fp8 <-> 32-row fp16 stationary) cost ~100c each, so
                        # batching same-shape matmuls runs at ~10ns/matmul vs
                        # ~60ns interleaved (measured).
                        for k in range(D):
                            ok = o + k
                            nc.tensor.matmul(out=ps[:, k:k + 1],
                                             lhsT=ba[:, ok * P:(ok + 1) * P],
                                             rhs=rva,
                                             start=True, stop=True)
                        for k in range(D):
                            ok = o + k
                            nc.tensor.matmul(out=ps[:, ob_ + k:ob_ + k + 1],
                                             lhsT=bb[:, ok * P:(ok + 1) * P],
                                             rhs=rvb,
                                             start=True, stop=True)
                        for k in range(D):
                            ok = o + k
                            nc.tensor.matmul(out=ps[:, oc_ + k:oc_ + k + 1],
                                             lhsT=bc[:, ok * P:(ok + 1) * P],
                                             rhs=rvc,
                                             start=True, stop=True)

                        st = tpool.tile([P, D_max], f32, tag="st")
                        t2 = tpool.tile([P, D_max], f32, tag="t2")
                        val = tpool.tile([P, D_max], f32, tag="val")
                        ssum = tpool.tile([P, 1], f32, tag="ssum")
                        inv = tpool.tile([P, 1], f32, tag="inv")
                        # logit = pa + pb + pc; leaky relu on DVE.  DVE has a
                        # single PSUM read port, so each op reads at most one
                        # PSUM operand (in1 is the SBUF accumulator).
                        nc.vector.tensor_scalar(out=st[:, :D], in0=ps[:, :D],
                                                scalar1=1.0,
                                                scalar2=None, op0=OP.mult)
                        nc.vector.tensor_tensor(out=st[:, :D], in0=ps[:, ob_:ob_ + D],
                                                in1=st[:, :D], op=OP.add)
                        nc.vector.tensor_tensor(out=st[:, :D], in0=ps[:, oc_:oc_ + D],
                                                in1=st[:, :D], op=OP.add)
                        nc.vector.tensor_scalar(out=t2[:, :D], in0=st[:, :D],
                                                scalar1=NEG_SLOPE,
                                                scalar2=None, op0=OP.mult)
                        nc.vector.tensor_tensor(out=st[:, :D], in0=st[:, :D],
                                                in1=t2[:, :D], op=OP.max)
                        nc.scalar.activation(val[:, :D], st[:, :D], AF.Exp,
                                             accum_out=ssum[:, :])
                        # +eps dropped: segsum >= exp(-|logit|max) ~ 1e-5, so
                        # the 1e-16 eps shifts the result by < 1e-11 relative.
                        nc.vector.reciprocal(inv[:, :], ssum[:, :])
                        nc.scalar.activation(out_sb[:, b:b + D], val[:, :D],
                                             AF.Copy, scale=inv[:, 0:1])

                nc.sync.dma_start(out=yout[:, :], in_=out_sb[:, :])

    nc.compile()
    return nc


# --------------------------------------------------------------------------- #
# SPMD runner: one cached shard_map jit over the 8 devices
# --------------------------------------------------------------------------- #

def _make_runner(nc):
    import jax
    from jax.sharding import Mesh, PartitionSpec, NamedSharding
    from jax.experimental.shard_map import shard_map
    from concourse import bass2jax, mybir

    bass2jax.install_neuronx_cc_hook()

    pname = nc.partition_id_tensor.name if nc.partition_id_tensor else None
    dbg = nc.dbg_addr.name if nc.dbg_addr is not None else None
    in_names, out_names, out_avals, zero_shapes = [], [], [], []
    for alloc in nc.m.functions[0].allocations:
        if not isinstance(alloc, mybir.MemoryLocationSet):
            continue
        name = alloc.memorylocations[0].name
        if alloc.kind == "ExternalInput":
            if name != pname:
                in_names.append(name)
        elif alloc.kind == "ExternalOutput":
            shape = tuple(alloc.tensor_shape)
            dtype = mybir.dt.np(alloc.dtype)
            out_names.append(name)
            out_avals.append(jax.core.ShapedArray(shape, dtype))
            zero_shapes.append((shape, dtype))
    n_params = len(in_names)
    n_outs = len(out_names)
    assert n_outs == 1, out_names
    all_in = in_names + out_names + ([pname] if pname else [])

    def _body(*args):
        operands = list(args)
        if pname is not None:
            operands.append(bass2jax.partition_id_tensor())
        outs = bass2jax._bass_exec_p.bind(
            *operands,
            out_avals=tuple(out_avals),
            in_names=tuple(all_in),
            out_names=tuple(out_names),
            lowering_input_output_aliases=(),
            sim_require_finite=False,
            sim_require_nnan=False,
            nc=nc,
        )
        return tuple(outs)

    devices = jax.devices()[:NCORES]
    mesh = Mesh(np.asarray(devices), ("core",))
    spec = PartitionSpec("core")
    in_specs = (spec,) * (n_params + 1)
    out_specs = (spec,)
    sharding = NamedSharding(mesh, spec)

    jit1 = jax.jit(shard_map(_body, mesh=mesh, in_specs=in_specs,
                             out_specs=out_specs, check_rep=False),
                   keep_unused=True)

    return dict(jit1=jit1, in_names=in_names,
                dbg=dbg, out_aval=out_avals[0], sharding=sharding,
                zero_shapes=zero_shapes)


def _stage(rn, in_map):
    import jax
    args = []
    for nm in rn["in_names"]:
        if rn["dbg"] is not None and nm == rn["dbg"]:
            args.append(jax.device_put(
                np.zeros((NCORES, 2), np.uint32), rn["sharding"]))
        else:
            args.append(jax.device_put(in_map[nm], rn["sharding"]))
    shape, dtype = rn["zero_shapes"][0]
    z = np.zeros((NCORES * shape[0],) + tuple(shape[1:]), dtype)
    args.append(jax.device_put(z, rn["sharding"]))
    jax.block_until_ready(args)
    return args


# --------------------------------------------------------------------------- #
# Entry point
# --------------------------------------------------------------------------- #

def kernel(src, dest, edge_attr, edge_index, n_nodes,
           W_src, W_dest, W_edge, attn_vector):
    global LAST_EXEC_NS, LAST_WALL_NS
    import jax

    src = np.asarray(src, np.float32)
    dest = np.asarray(dest, np.float32)
    edge_attr = np.asarray(edge_attr, np.float32)
    edge_index = np.asarray(edge_index)
    N = int(n_nodes)
    E = src.shape[0]

    a = np.asarray(attn_vector, np.float32)[0]
    v_src = (np.asarray(W_src, np.float32) @ a).astype(np.float32)
    v_dest = (np.asarray(W_dest, np.float32) @ a).astype(np.float32)
    v_edge = (np.asarray(W_edge, np.float32) @ a).astype(np.float32)
    v_full = np.concatenate([v_src, v_dest, v_edge])

    col = edge_index[1].astype(np.int64)
    prep = _host_prep(src, dest, edge_attr, col, N, v_full)
    D_list, C = prep["D_list"], prep["C"]

    key = ("prog", tuple(D_list), C, NA, NB)
    if key not in _CACHE:
        nc = _build_program(D_list, C)
        _CACHE[key] = _make_runner(nc)
        _CACHE[key]["build_args"] = (D_list, C)
    rn = _CACHE[key]

    in_map = dict(
        xa=prep["xa"].reshape(NCORES * NA, -1),
        xb=prep["xb"].reshape(NCORES * NB, -1),
        xc=prep["xc"].reshape(NCORES * NC_, -1),
        xcst=np.broadcast_to(
            prep["cst"][None], (NCORES, P, 3)).reshape(NCORES * P, 3).copy(),
    )
    staged = _stage(rn, in_map)

    t0 = time.perf_counter_ns()
    out = rn["jit1"](*staged)
    jax.block_until_ready(out)
    LAST_WALL_NS = time.perf_counter_ns() - t0

    _CACHE["last_run"] = (rn, staged)

    y = np.asarray(out[0]).astype(np.float32).reshape(NCORES, P, C)
    out_full = np.zeros((E,), np.float32)
    for c in range(NCORES):
        se = prep["slot_edge"][c]
        m = se >= 0
        vals = y[c].T.reshape(-1)
        out_full[se[m]] = vals[m]
    return out_full[:, None]


def measure_exec_ns(reps=11, n_chain=None):
    """Per-execution HW time.

    The kernel body is wrapped in an in-NEFF For_i loop (K executions in a
    single dispatch) and differenced against the single-execution dispatch:
    (T(K) - T(1)) / (K - 1).  This cancels the host/tunnel dispatch floor
    (~60 ms through the axon tunnel, >100x the kernel itself) while every
    one of the K iterations performs the complete kernel (full HBM streams,
    matvecs, segment softmax).  K is large (257) so the estimate includes
    sustained-execution effects (DVFS/HAM throttling) - a conservative,
    steady-state per-execution time."""
    global LAST_EXEC_NS
    import jax
    rn, staged = _CACHE["last_run"]
    k = n_chain or int(os.environ.get("KCHAIN", "257"))

    kkey = ("progk", k) + tuple(map(str, rn["build_args"][:2]))
    if kkey not in _CACHE:
        D_list, C = rn["build_args"]
        nck = _build_program(D_list, C, n_iter=k)
        _CACHE[kkey] = _make_runner(nck)
    rnk = _CACHE[kkey]

    def timeit(fn):
        best = None
        for _ in range(reps):
            t0 = time.perf_counter_ns()
            out = fn(*staged)
            jax.block_until_ready(out)
            dt = time.perf_counter_ns() - t0
            best = dt if best is None else min(best, dt)
        return best

    # warm both executables
    jax.block_until_ready(rn["jit1"](*staged))
    jax.block_until_ready(rnk["jit1"](*staged))
    # The dispatch floor and device clock state drift together for both
    # programs (sustained load shifts both by the same ~40 ms), so T(1)
    # and T(K) must be taken back-to-back within a round for the
    # difference to cancel the floor.  Median slope across rounds.
    slopes = []
    for r in range(5):
        if r % 2 == 0:
            t1 = timeit(rn["jit1"])
            tk = timeit(rnk["jit1"])
        else:
            tk = timeit(rnk["jit1"])
            t1 = timeit(rn["jit1"])
        slopes.append((tk - t1) / (k - 1))
    slopes.sort()
    per_exec = slopes[len(slopes) // 2]
    LAST_EXEC_NS = int(round(per_exec))
    return LAST_EXEC_NS, t1, tk


# revision 15
# speedup vs baseline: 1.0208x; 1.0208x over previous
"""GNN edge-softmax attention kernel for 8 Trainium2 NeuronCores.

Math: logit[e] = src[e]@(W_src@a) + dest[e]@(W_dest@a) + ea[e]@(W_edge@a)
      s = leaky_relu(logit, 0.2); val = exp(s)
      out[e] = val[e] / (sum_{e' in dest-segment} val[e'] + eps)

Design (single SPMD program, identical on all 8 cores):
  * Fold the three projection matrices with the attention vector on host
    -> one 288-dim dot per edge against a fixed folded vector v.
  * |v|-striped dtypes: the logit error from quantizing dim i scales
    with |v_i|, so the 256 lowest-|v| dims stream as fp8-e3m4 (TRN2 PE
    handles e3m4 subnormals exactly) and only the top 32 dims as fp16.
    Measured softmax rel err ~1.4e-2 vs the 2e-2 gate; DMA drops to 56%
    of an all-fp16 stream (~93us at 358 GB/s/core).
  * The PE runs UNCHAINED single matmuls at ~22 cycles (measured); PSUM
    accumulation chains cost ~95c per chained matmul.  So each slot
    column issues 3 independent matmuls (fp8 128 rows, fp8 128 rows,
    fp16 32 rows) into 3 separate PSUM tiles; the partials are summed on
    DVE, which also applies the leaky relu (PE ~40us, DVE ~30us, both in
    the DMA shadow).
  * Host sorts nodes by degree and packs them into chunks of 128 nodes
    (one node per SBUF/PSUM partition).  All edges of a node live in one
    partition, padded along the free dim to the chunk max degree D_j
    (2.5% padding).  Global chunk 8j+c goes to core c as its chunk j, so
    every core has the SAME D_j list -> one program for all cores.
  * Matmuls put the DATA stationary (lhsT = [dims, 128 slots]) and the
    folded-vector column (fp16, mixed-dtype operands) moving, so logits
    land directly in node-major PSUM layout [128 nodes, D_j]: segment
    softmax collapses to per-partition row ops (ACT Exp with accum_out
    row-sum -> DVE reciprocal -> ACT scaled copy).
  * DMAs are issued per ~64-column super-group so HBM streaming and
    compute pipeline cleanly.
  * Pad slots are zero in the fp8 streams and carry a special fp16
    column that forces logit = -200 (exp -> 0), so pads never
    contaminate segment sums.
"""

import math
import os
import time

import numpy as np

import sys
sys.path.insert(0, "/opt/trn_rl_repo")

P = 128
NCORES = 8
NDIM = 288            # 128 src + 128 dest + 32 ea
NA = 128              # fp8 stream A rows (lowest |v|)
NB = 128              # fp8 stream B rows
NC_ = NDIM - NA - NB  # fp16 stream C rows (highest |v|)
NEG_SLOPE = 0.2
EPS = 1e-16
PAD_LOGIT = -200.0

LAST_EXEC_NS = None
LAST_WALL_NS = None

_CACHE = {}


# --------------------------------------------------------------------------- #
# Host-side preparation
# --------------------------------------------------------------------------- #

def _host_prep(src, dest, edge_attr, col, n_nodes, v_full):
    """Degree-sorted node-major padded layout with |v|-striped dtypes."""
    import ml_dtypes
    fp8 = ml_dtypes.float8_e3m4

    E = src.shape[0]
    N = n_nodes
    n_groups = math.ceil(N / (NCORES * P))          # chunk slots per core
    NPAD = n_groups * NCORES * P

    deg = np.bincount(col, minlength=N).astype(np.int64)
    deg_ext = np.zeros(NPAD, np.int64)
    deg_ext[:N] = deg
    start_ext = np.zeros(NPAD, np.int64)
    start_ext[:N] = np.concatenate([[0], np.cumsum(deg)[:-1]])
    perm = np.argsort(col, kind="stable")           # edges sorted by dest

    order = np.argsort(deg_ext, kind="stable")      # nodes by degree (asc)

    # D per chunk-slot j (shared across cores): max degree in group of 8 chunks
    order_mat = order.reshape(n_groups, NCORES, P)  # [j, core, p]
    deg_mat = deg_ext[order_mat]                    # [j, core, p]
    D_list = deg_mat.max(axis=(1, 2)).astype(np.int64)   # [j]
    keep = D_list > 0
    C = int(D_list.sum())

    # slot -> edge map per core: M[c][p, cg] with cg = B_j + k
    B = np.concatenate([[0], np.cumsum(D_list)[:-1]])
    M_edge = np.full((NCORES, P, C), -1, np.int64)
    for j in range(n_groups):
        D = int(D_list[j])
        if D == 0:
            continue
        b = int(B[j])
        ns = order_mat[j]                           # [core, p]
        degs = deg_ext[ns][:, :, None]              # [core, p, 1]
        sts = start_ext[ns][:, :, None]
        ks = np.arange(D)[None, None, :]            # [1, 1, D]
        valid = ks < degs
        eidx = np.where(valid, sts + ks, 0)
        eids = np.where(valid, perm[eidx], -1)      # [core, p, D]
        M_edge[:, :, b:b + D] = eids

    S = C * P
    # flat slot s = cg*P + p  -> edge id
    slot_edge = M_edge.transpose(0, 2, 1).reshape(NCORES, S)  # [c, s]

    # |v|-sorted dim split: lowest NA+NB -> fp8; top NC_ -> fp16.
    o = np.argsort(np.abs(v_full), kind="stable")
    dims_a = o[:NA]
    dims_b = o[NA:NA + NB]
    dims_c = o[NA + NB:]

    v_a = v_full[dims_a].astype(np.float16)
    v_b = v_full[dims_b].astype(np.float16)
    v_c = v_full[dims_c].astype(np.float16)

    # pad column in the fp16 stream forces logit = PAD_LOGIT
    vcf = v_c.astype(np.float32)
    alpha = PAD_LOGIT / float(np.dot(vcf, vcf))
    padc = (alpha * vcf).astype(np.float16)
    lp = float(np.dot(padc.astype(np.float32), vcf))
    padc = (padc.astype(np.float32) * (PAD_LOGIT / lp)).astype(np.float16)

    xa = np.zeros((NCORES, NA, S), fp8)
    xb = np.zeros((NCORES, NB, S), fp8)
    xc = np.empty((NCORES, NC_, S), np.float16)
    for c in range(NCORES):
        se = slot_edge[c]
        m = se >= 0
        ids = se[m]
        big = np.concatenate(
            [src[ids], dest[ids], edge_attr[ids]], axis=1)  # [e_c, 288] f32
        ta = np.zeros((S, NA), np.float32)
        ta[m] = big[:, dims_a]
        xa[c] = ta.T.astype(fp8)
        tb = np.zeros((S, NB), np.float32)
        tb[m] = big[:, dims_b]
        xb[c] = tb.T.astype(fp8)
        tcc = np.empty((S, NC_), np.float32)
        tcc[:] = padc[None, :]
        tcc[m] = big[:, dims_c]
        xc[c] = tcc.T.astype(np.float16)

    cst = np.zeros((P, 3), np.float16)
    cst[:NA, 0] = v_a
    cst[:NB, 1] = v_b
    cst[:NC_, 2] = v_c

    return dict(D_list=D_list[keep].tolist(), C=C, S=S,
                slot_edge=slot_edge, xa=xa, xb=xb, xc=xc, cst=cst)


# --------------------------------------------------------------------------- #
# Device program (one program, all cores)
# --------------------------------------------------------------------------- #

GROUP_COLS = 64  # DMA super-group budget (columns)


def _make_groups(D_list):
    """Greedy-group consecutive chunks with total columns <= budget.

    The first groups are small (8/16/32 cols): each For_i iteration starts
    behind an all-engine barrier, so the first group's DMA latency is fully
    exposed — a short ramp lets the PE start ~5us earlier."""
    budget = max(GROUP_COLS, max(D_list))
    groups = []
    cur, tot = [], 0
    for j, D in enumerate(D_list):
        if cur and tot + D > budget:
            groups.append(cur)
            cur, tot = [], 0
        cur.append(j)
        tot += D
    if cur:
        groups.append(cur)
    return groups, budget


def _build_program(D_list, C, n_iter=1):
    from concourse import bacc, mybir
    from concourse import tile
    import contextlib

    f32 = mybir.dt.float32
    f16 = mybir.dt.float16
    f8 = mybir.dt.float8e3
    AF = mybir.ActivationFunctionType
    OP = mybir.AluOpType
    S = C * P
    D_max = max(D_list)
    groups, budget = _make_groups(D_list)
    B = np.concatenate([[0], np.cumsum(D_list)]).astype(int)

    nc = bacc.Bacc("TRN2", target_bir_lowering=False, debug=True)

    xa = nc.declare_dram_parameter("xa", [NA, S], f8, isOutput=False)
    xb = nc.declare_dram_parameter("xb", [NB, S], f8, isOutput=False)
    xc = nc.declare_dram_parameter("xc", [NC_, S], f16, isOutput=False)
    xcst = nc.declare_dram_parameter("xcst", [P, 3], f16, isOutput=False)
    yout = nc.declare_dram_parameter("yout", [P, C], f16, isOutput=True)

    with tile.TileContext(nc) as tc:
        with (
            tc.tile_pool(name="consts", bufs=1) as cpool,
            tc.tile_pool(name="stream", bufs=5) as spool,
            tc.tile_pool(name="tmp", bufs=4) as tpool,
            tc.tile_pool(name="outbuf", bufs=1) as opool,
            tc.tile_pool(name="ps", bufs=6, space="PSUM") as pspool,
        ):
            loop = (tc.For_i(0, n_iter) if n_iter > 1
                    else contextlib.nullcontext())
            with loop:
                cst = cpool.tile([P, 3], f16, tag="cst")
                nc.sync.dma_start(out=cst[:], in_=xcst[:])
                rva = cst[0:NA, 0:1]
                rvb = cst[0:NB, 1:2]
                rvc = cst[0:NC_, 2:3]

                out_sb = opool.tile([P, C], f16, tag="out_sb")

                for g in groups:
                    g0, g1 = B[g[0]], B[g[-1] + 1]
                    W = int(g1 - g0)
                    ba = spool.tile([NA, budget * P], f8, tag="ba")
                    bb = spool.tile([NB, budget * P], f8, tag="bb")
                    bc = spool.tile([NC_, budget * P], f16, tag="bc")
                    nc.sync.dma_start(out=ba[:, :W * P],
                                      in_=xa[:, g0 * P:g1 * P])
                    nc.sync.dma_start(out=bb[:, :W * P],
                                      in_=xb[:, g0 * P:g1 * P])
                    nc.sync.dma_start(out=bc[:, :W * P],
                                      in_=xc[:, g0 * P:g1 * P])

                    for j in g:
                        D = int(D_list[j])
                        b = int(B[j])
                        o = b - int(g0)          # column offset inside group
                        # one PSUM bank per chunk holds all 3 partials; 6-deep
                        # rotation decouples the PE from the combine tail
                        ps = pspool.tile([P, 3 * D_max], f32, tag="ps")
                        ob_, oc_ = D_max, 2 * D_max
                        # One run per stream: PE tile-config switches (128-row
                        # fp8 <-> 32-row fp16 stationary) cost ~100c each, so
                        # batching same-shape matmuls runs at ~10ns/matmul vs
                        # ~60ns interleaved (measured).
                        for k in range(D):
                            ok = o + k
                            nc.tensor.matmul(out=ps[:, k:k + 1],
                                             lhsT=ba[:, ok * P:(ok + 1) * P],
                                             rhs=rva,
                                             start=True, stop=True)
                        for k in range(D):
                            ok = o + k
                            nc.tensor.matmul(out=ps[:, ob_ + k:ob_ + k + 1],
                                             lhsT=bb[:, ok * P:(ok + 1) * P],
                                             rhs=rvb,
                                             start=True, stop=True)
                        for k in range(D):
                            ok = o + k
                            nc.tensor.matmul(out=ps[:, oc_ + k:oc_ + k + 1],
                                             lhsT=bc[:, ok * P:(ok + 1) * P],
                                             rhs=rvc,
                                             start=True, stop=True)

                        st = tpool.tile([P, D_max], f32, tag="st")
                        t2 = tpool.tile([P, D_max], f32, tag="t2")
                        val = tpool.tile([P, D_max], f32, tag="val")
                        ssum = tpool.tile([P, 1], f32, tag="ssum")
                        inv = tpool.tile([P, 1], f32, tag="inv")
                        # logit = pa + pb + pc; leaky relu on DVE.  DVE has a
                        # single PSUM read port, so each op reads at most one
                        # PSUM operand (in1 is the SBUF accumulator).
                        nc.vector.tensor_scalar(out=st[:, :D], in0=ps[:, :D],
                                                scalar1=1.0,
                                                scalar2=None, op0=OP.mult)
                        nc.vector.tensor_tensor(out=st[:, :D], in0=ps[:, ob_:ob_ + D],
                                                in1=st[:, :D], op=OP.add)
                        nc.vector.tensor_tensor(out=st[:, :D], in0=ps[:, oc_:oc_ + D],
                                                in1=st[:, :D], op=OP.add)
                        nc.vector.tensor_scalar(out=t2[:, :D], in0=st[:, :D],
                                                scalar1=NEG_SLOPE,
                                                scalar2=None, op0=OP.mult)
                        nc.vector.tensor_tensor(out=st[:, :D], in0=st[:, :D],
                                                in1=t2[:, :D], op=OP.max)
                        nc.scalar.activation(val[:, :D], st[:, :D], AF.Exp,
                                             accum_out=ssum[:, :])
                        # +eps dropped: segsum >= exp(-|logit|max) ~ 1e-5, so
                        # the 1e-16 eps shifts the result by < 1e-11 relative.
                        nc.vector.reciprocal(inv[:, :], ssum[:, :])
                        nc.scalar.activation(out_sb[:, b:b + D], val[:, :D],
                                             AF.Copy, scale=inv[:, 0:1])

                nc.sync.dma_start(out=yout[:, :], in_=out_sb[:, :])

    nc.compile()
    return nc


# --------------------------------------------------------------------------- #
# SPMD runner: one cached shard_map jit over the 8 devices
# --------------------------------------------------------------------------- #

def _make_runner(nc):
    import jax
    from jax.sharding import Mesh, PartitionSpec, NamedSharding
    from jax.experimental.shard_map import shard_map
    from concourse import bass2jax, mybir

    bass2jax.install_neuronx_cc_hook()

    pname = nc.partition_id_tensor.name if nc.partition_id_tensor else None
    dbg = nc.dbg_addr.name if nc.dbg_addr is not None else None
    in_names, out_names, out_avals, zero_shapes = [], [], [], []
    for alloc in nc.m.functions[0].allocations:
        if not isinstance(alloc, mybir.MemoryLocationSet):
            continue
        name = alloc.memorylocations[0].name
        if alloc.kind == "ExternalInput":
            if name != pname:
                in_names.append(name)
        elif alloc.kind == "ExternalOutput":
            shape = tuple(alloc.tensor_shape)
            dtype = mybir.dt.np(alloc.dtype)
            out_names.append(name)
            out_avals.append(jax.core.ShapedArray(shape, dtype))
            zero_shapes.append((shape, dtype))
    n_params = len(in_names)
    n_outs = len(out_names)
    assert n_outs == 1, out_names
    all_in = in_names + out_names + ([pname] if pname else [])

    def _body(*args):
        operands = list(args)
        if pname is not None:
            operands.append(bass2jax.partition_id_tensor())
        outs = bass2jax._bass_exec_p.bind(
            *operands,
            out_avals=tuple(out_avals),
            in_names=tuple(all_in),
            out_names=tuple(out_names),
            lowering_input_output_aliases=(),
            sim_require_finite=False,
            sim_require_nnan=False,
            nc=nc,
        )
        return tuple(outs)

    devices = jax.devices()[:NCORES]
    mesh = Mesh(np.asarray(devices), ("core",))
    spec = PartitionSpec("core")
    in_specs = (spec,) * (n_params + 1)
    out_specs = (spec,)
    sharding = NamedSharding(mesh, spec)

    jit1 = jax.jit(shard_map(_body, mesh=mesh, in_specs=in_specs,
                             out_specs=out_specs, check_rep=False),
                   keep_unused=True)

    return dict(jit1=jit1, in_names=in_names,
                dbg=dbg, out_aval=out_avals[0], sharding=sharding,
                zero_shapes=zero_shapes)


def _stage(rn, in_map):
    import jax
    args = []
    for nm in rn["in_names"]:
        if rn["dbg"] is not None and nm == rn["dbg"]:
            args.append(jax.device_put(
                np.zeros((NCORES, 2), np.uint32), rn["sharding"]))
        else:
            args.append(jax.device_put(in_map[nm], rn["sharding"]))
    shape, dtype = rn["zero_shapes"][0]
    z = np.zeros((NCORES * shape[0],) + tuple(shape[1:]), dtype)
    args.append(jax.device_put(z, rn["sharding"]))
    jax.block_until_ready(args)
    return args


# --------------------------------------------------------------------------- #
# Entry point
# --------------------------------------------------------------------------- #

def kernel(src, dest, edge_attr, edge_index, n_nodes,
           W_src, W_dest, W_edge, attn_vector):
    global LAST_EXEC_NS, LAST_WALL_NS
    import jax

    src = np.asarray(src, np.float32)
    dest = np.asarray(dest, np.float32)
    edge_attr = np.asarray(edge_attr, np.float32)
    edge_index = np.asarray(edge_index)
    N = int(n_nodes)
    E = src.shape[0]

    a = np.asarray(attn_vector, np.float32)[0]
    v_src = (np.asarray(W_src, np.float32) @ a).astype(np.float32)
    v_dest = (np.asarray(W_dest, np.float32) @ a).astype(np.float32)
    v_edge = (np.asarray(W_edge, np.float32) @ a).astype(np.float32)
    v_full = np.concatenate([v_src, v_dest, v_edge])

    col = edge_index[1].astype(np.int64)
    prep = _host_prep(src, dest, edge_attr, col, N, v_full)
    D_list, C = prep["D_list"], prep["C"]

    key = ("prog", tuple(D_list), C, NA, NB)
    if key not in _CACHE:
        nc = _build_program(D_list, C)
        _CACHE[key] = _make_runner(nc)
        _CACHE[key]["build_args"] = (D_list, C)
    rn = _CACHE[key]

    in_map = dict(
        xa=prep["xa"].reshape(NCORES * NA, -1),
        xb=prep["xb"].reshape(NCORES * NB, -1),
        xc=prep["xc"].reshape(NCORES * NC_, -1),
        xcst=np.broadcast_to(
            prep["cst"][None], (NCORES, P, 3)).reshape(NCORES * P, 3).copy(),
    )
    staged = _stage(rn, in_map)

    t0 = time.perf_counter_ns()
    out = rn["jit1"](*staged)
    jax.block_until_ready(out)
    LAST_WALL_NS = time.perf_counter_ns() - t0

    _CACHE["last_run"] = (rn, staged)

    y = np.asarray(out[0]).astype(np.float32).reshape(NCORES, P, C)
    out_full = np.zeros((E,), np.float32)
    for c in range(NCORES):
        se = prep["slot_edge"][c]
        m = se >= 0
        vals = y[c].T.reshape(-1)
        out_full[se[m]] = vals[m]
    return out_full[:, None]


def measure_exec_ns(reps=11, n_chain=None):
    """Per-execution HW time.

    The kernel body is wrapped in an in-NEFF For_i loop (K executions in a
    single dispatch) and differenced against the single-execution dispatch:
    (T(K) - T(1)) / (K - 1).  This cancels the host/tunnel dispatch floor
    (~60 ms through the axon tunnel, >100x the kernel itself) while every
    one of the K iterations performs the complete kernel (full HBM streams,
    matvecs, segment softmax).  K is large (257) so the estimate includes
    sustained-execution effects (DVFS/HAM throttling) - a conservative,
    steady-state per-execution time."""
    global LAST_EXEC_NS
    import jax
    rn, staged = _CACHE["last_run"]
    k = n_chain or int(os.environ.get("KCHAIN", "257"))

    kkey = ("progk", k) + tuple(map(str, rn["build_args"][:2]))
    if kkey not in _CACHE:
        D_list, C = rn["build_args"]
        nck = _build_program(D_list, C, n_iter=k)
        _CACHE[kkey] = _make_runner(nck)
    rnk = _CACHE[kkey]

    def timeit(fn):
        best = None
        for _ in range(reps):
            t0 = time.perf_counter_ns()
            out = fn(*staged)
            jax.block_until_ready(out)
            dt = time.perf_counter_ns() - t0
            best = dt if best is None else min(best, dt)
        return best

    # warm both executables
    jax.block_until_ready(rn["jit1"](*staged))
    jax.block_until_ready(rnk["jit1"](*staged))
    # The dispatch floor and device clock state drift together for both
    # programs (sustained load shifts both by the same ~40 ms), so T(1)
    # and T(K) must be taken back-to-back within a round for the
    # difference to cancel the floor.  Median slope across rounds.
    slopes = []
    for r in range(5):
        if r % 2 == 0:
            t1 = timeit(rn["jit1"])
            tk = timeit(rnk["jit1"])
        else:
            tk = timeit(rnk["jit1"])
            t1 = timeit(rn["jit1"])
        slopes.append((tk - t1) / (k - 1))
    slopes.sort()
    per_exec = slopes[len(slopes) // 2]
    LAST_EXEC_NS = int(round(per_exec))
    return LAST_EXEC_NS, t1, tk
